# revision 7
# baseline (speedup 1.0000x reference)
"""Trainium2 Bass kernel for nn_ConvEnhanced_65481071405356.

The reference op is a handful of scalar reductions on an 8x8 input:

    d = data
    for i, k in enumerate([2, 3, 5, 7]):
        d = resize(d, k)          # crop to 2x2 at i=0, then zero-pad after
        logit_i = sum(d * dw_k) * pw_w[i] + pw_b[i]
        out_i = sigmoid(logit_i)
        attn_i = mean(softmax((d * attn_w[i]).ravel()))
    result = (mean(out) + d.mean()) * mean(attn)

Two exact algebraic facts collapse this:
  * After the first crop, d's nonzero support is always data[:2, :2], so only
    the top-left 2x2 of data and of each dw_k ever contribute, and the final
    d.mean() is sum(data[:2, :2]) / 49.
  * mean(softmax(x)) over n elements is exactly 1/n (softmax sums to 1), so
    the attn branch is the constant A = (1/4 + 1/9 + 1/25 + 1/49) / 4.

So:  result = (A/4) * sum_i sigmoid(s_i * pw_w[i] + pw_b[i]) + (A/49) * sum(d2)
with s_i = <data[:2,:2], dw_k[:2,:2]> and d2 = data[:2,:2].

Device kernel (replicated on all 8 cores; the op is scalar-sized so there is
nothing to shard): the host packs all operands and constants into a single
[8, 12] f32 buffer so the device needs exactly one input DMA:

    row i in 0..3:  [dw_i 2x2 (4) | d2 (4) | pw_w[i] | pw_b[i] | 0    | A/4 ]
    row 4+j:        [0 ...                                      | d2_j | A/49]

  1. tensor_tensor_reduce (DVE): s[4,1] = sum_x(T[0:4,0:4] * T[0:4,4:8])
  2. activation Sigmoid (ACT):   T[0:4,10] = sigmoid(s * pw_w + pw_b)
                                 (per-partition scale/bias APs)
  3. matmul (PE, K=8):           psum[1,1] = T[:,10].T @ T[:,11]
                                 = A/4 * sum(sig) + A/49 * sum(d2)  == result
  4. copy PSUM -> SBUF, DMA out.
"""

import sys

import numpy as np

if "/opt/trn_rl_repo" not in sys.path:
    sys.path.insert(0, "/opt/trn_rl_repo")

import concourse.mybir as mybir
from concourse import bacc, tile
from concourse.bass_utils import run_bass_kernel_spmd

N_CORES = 8
_F32 = mybir.dt.float32

# mean(softmax(x)) over k*k elements == 1/k^2 exactly; mean over the 4 steps.
ATTN_MEAN = (1 / 4 + 1 / 9 + 1 / 25 + 1 / 49) / 4

_NC_CACHE = None


def build_bass():
    nc = bacc.Bacc(None)
    packed = nc.dram_tensor("packed", [8, 12], _F32, kind="ExternalInput")
    out = nc.dram_tensor("out", [1, 1], _F32, kind="ExternalOutput")
    with tile.TileContext(nc) as tc:
        with (
            tc.tile_pool(name="sb", bufs=1) as sb,
            tc.tile_pool(name="ps", bufs=1, space="PSUM") as ps,
        ):
            T = sb.tile([8, 12], _F32)
            prod = sb.tile([4, 4], _F32)
            s = sb.tile([4, 1], _F32)
            res = sb.tile([1, 1], _F32)
            P = ps.tile([1, 1], _F32)

            nc.gpsimd.dma_start(T[:, :], packed[:, :])
            nc.vector.scalar_tensor_tensor(
                out=prod[:, :],
                in0=T[0:4, 0:4],
                scalar=1.0,
                in1=T[0:4, 4:8],
                op0=mybir.AluOpType.mult,
                op1=mybir.AluOpType.mult,
                accum_out=s[:, :],
            )
            nc.scalar.activation(
                T[0:4, 10:11],
                s[:, :],
                mybir.ActivationFunctionType.Sigmoid,
                bias=T[0:4, 9:10],
                scale=T[0:4, 8:9],
            )
            nc.tensor.matmul(P[:, :], T[:, 10:11], T[:, 11:12], start=True, stop=True)
            nc.vector.tensor_copy(res[:, :], P[:, :])
            nc.gpsimd.dma_start(out[:, :], res[:, :])
    if not nc.is_finalized():
        nc.finalize()
    return nc


def pack_inputs(data, dw2, dw3, dw5, dw7, pw_w, pw_b):
    d2 = np.asarray(data, np.float32)[:2, :2].reshape(-1)
    packed = np.zeros((8, 12), np.float32)
    for i, w in enumerate((dw2, dw3, dw5, dw7)):
        packed[i, 0:4] = np.asarray(w, np.float32)[:2, :2].reshape(-1)
    packed[0:4, 4:8] = d2
    packed[0:4, 8] = np.asarray(pw_w, np.float32)
    packed[0:4, 9] = np.asarray(pw_b, np.float32)
    packed[0:4, 11] = np.float32(ATTN_MEAN / 4)
    packed[4:8, 10] = d2
    packed[4:8, 11] = np.float32(ATTN_MEAN / 49)
    return packed


def run_packed(packed, **spmd_kwargs):
    global _NC_CACHE
    if _NC_CACHE is None:
        _NC_CACHE = build_bass()
    in_maps = [{"packed": packed} for _ in range(N_CORES)]
    return run_bass_kernel_spmd(
        _NC_CACHE, in_maps, core_ids=list(range(N_CORES)), **spmd_kwargs
    )


def kernel(data, dw2, dw3, dw5, dw7, pw_w, pw_b, attn_w):
    packed = pack_inputs(data, dw2, dw3, dw5, dw7, pw_w, pw_b)
    r = run_packed(packed)
    return np.asarray(r.results[0]["out"][0, 0], dtype=np.float32)


# revision 8
# speedup vs baseline: 1.0778x; 1.0778x over previous
"""Trainium2 Bass kernel for nn_ConvEnhanced_65481071405356.

The reference op is a handful of scalar reductions on an 8x8 input:

    d = data
    for i, k in enumerate([2, 3, 5, 7]):
        d = resize(d, k)          # crop to 2x2 at i=0, then zero-pad after
        logit_i = sum(d * dw_k) * pw_w[i] + pw_b[i]
        out_i = sigmoid(logit_i)
        attn_i = mean(softmax((d * attn_w[i]).ravel()))
    result = (mean(out) + d.mean()) * mean(attn)

Two exact algebraic facts collapse this:
  * After the first crop, d's nonzero support is always data[:2, :2], so only
    the top-left 2x2 of data and of each dw_k ever contribute, and the final
    d.mean() is sum(data[:2, :2]) / 49.
  * mean(softmax(x)) over n elements is exactly 1/n (softmax sums to 1), so
    the attn branch is the constant A = (1/4 + 1/9 + 1/25 + 1/49) / 4.

So:  result = (A/4) * sum_i sigmoid(s_i * pw_w[i] + pw_b[i]) + (A/49) * sum(d2)
with s_i = <data[:2,:2], dw_k[:2,:2]> and d2 = data[:2,:2].

Device kernel (replicated on all 8 cores; the op is scalar-sized so there is
nothing to shard): the host packs all operands and constants into a single
[8, 12] f32 buffer so the device needs exactly one input DMA:

    row i in 0..3:  [dw_i 2x2 (4) | d2 (4) | pw_w[i] | pw_b[i] | 0    | A/4 ]
    row 4+j:        [0 ...                                      | d2_j | A/49]

  1. tensor_tensor_reduce (DVE): s[4,1] = sum_x(T[0:4,0:4] * T[0:4,4:8])
  2. activation Sigmoid (ACT):   T[0:4,10] = sigmoid(s * pw_w + pw_b)
                                 (per-partition scale/bias APs)
  3. matmul (PE, K=8):           psum[1,1] = T[:,10].T @ T[:,11]
                                 = A/4 * sum(sig) + A/49 * sum(d2)  == result
  4. copy PSUM -> SBUF, DMA out.
"""

import sys

import numpy as np

if "/opt/trn_rl_repo" not in sys.path:
    sys.path.insert(0, "/opt/trn_rl_repo")

import concourse.mybir as mybir
from concourse import bacc, tile
from concourse.bass_utils import run_bass_kernel_spmd

N_CORES = 8
_F32 = mybir.dt.float32

# mean(softmax(x)) over k*k elements == 1/k^2 exactly; mean over the 4 steps.
ATTN_MEAN = (1 / 4 + 1 / 9 + 1 / 25 + 1 / 49) / 4

_NC_CACHE = None


def build_bass():
    nc = bacc.Bacc(None)
    packed = nc.dram_tensor("packed", [8, 12], _F32, kind="ExternalInput")
    out = nc.dram_tensor("out", [1, 1], _F32, kind="ExternalOutput")
    with tile.TileContext(nc) as tc:
        with (
            tc.tile_pool(name="sb", bufs=1) as sb,
            tc.tile_pool(name="ps", bufs=1, space="PSUM") as ps,
        ):
            T = sb.tile([8, 12], _F32)
            prod = sb.tile([4, 4], _F32)
            s = sb.tile([4, 1], _F32)
            res = sb.tile([1, 1], _F32)
            P = ps.tile([1, 1], _F32)

            # Dependency-free sigmoid on a preamble-initialized const AP:
            # pulls the ACT-table load to the head of the Scalar queue so it
            # overlaps the input DMA instead of sitting on the critical path.
            dummy = sb.tile([1, 1], _F32)
            nc.scalar.activation(
                dummy[:, :],
                nc.const_aps.scalar_like(0.0, dummy[:, :]),
                mybir.ActivationFunctionType.Sigmoid,
            )

            nc.gpsimd.dma_start(T[:, :], packed[:, :])
            nc.vector.scalar_tensor_tensor(
                out=prod[:, :],
                in0=T[0:4, 0:4],
                scalar=1.0,
                in1=T[0:4, 4:8],
                op0=mybir.AluOpType.mult,
                op1=mybir.AluOpType.mult,
                accum_out=s[:, :],
            )
            nc.scalar.activation(
                T[0:4, 10:11],
                s[:, :],
                mybir.ActivationFunctionType.Sigmoid,
                bias=T[0:4, 9:10],
                scale=T[0:4, 8:9],
            )
            nc.tensor.matmul(P[:, :], T[:, 10:11], T[:, 11:12], start=True, stop=True)
            nc.vector.tensor_copy(res[:, :], P[:, :])
            nc.gpsimd.dma_start(out[:, :], res[:, :])
    if not nc.is_finalized():
        nc.finalize()
    return nc


def pack_inputs(data, dw2, dw3, dw5, dw7, pw_w, pw_b):
    d2 = np.asarray(data, np.float32)[:2, :2].reshape(-1)
    packed = np.zeros((8, 12), np.float32)
    for i, w in enumerate((dw2, dw3, dw5, dw7)):
        packed[i, 0:4] = np.asarray(w, np.float32)[:2, :2].reshape(-1)
    packed[0:4, 4:8] = d2
    packed[0:4, 8] = np.asarray(pw_w, np.float32)
    packed[0:4, 9] = np.asarray(pw_b, np.float32)
    packed[0:4, 11] = np.float32(ATTN_MEAN / 4)
    packed[4:8, 10] = d2
    packed[4:8, 11] = np.float32(ATTN_MEAN / 49)
    return packed


def run_packed(packed, **spmd_kwargs):
    global _NC_CACHE
    if _NC_CACHE is None:
        _NC_CACHE = build_bass()
    in_maps = [{"packed": packed} for _ in range(N_CORES)]
    return run_bass_kernel_spmd(
        _NC_CACHE, in_maps, core_ids=list(range(N_CORES)), **spmd_kwargs
    )


def kernel(data, dw2, dw3, dw5, dw7, pw_w, pw_b, attn_w):
    packed = pack_inputs(data, dw2, dw3, dw5, dw7, pw_w, pw_b)
    r = run_packed(packed)
    return np.asarray(r.results[0]["out"][0, 0], dtype=np.float32)


# revision 9
# speedup vs baseline: 1.1629x; 1.0790x over previous
"""Trainium2 Bass kernel for nn_ConvEnhanced_65481071405356.

The reference op is a handful of scalar reductions on an 8x8 input:

    d = data
    for i, k in enumerate([2, 3, 5, 7]):
        d = resize(d, k)          # crop to 2x2 at i=0, then zero-pad after
        logit_i = sum(d * dw_k) * pw_w[i] + pw_b[i]
        out_i = sigmoid(logit_i)
        attn_i = mean(softmax((d * attn_w[i]).ravel()))
    result = (mean(out) + d.mean()) * mean(attn)

Two exact algebraic facts collapse this:
  * After the first crop, d's nonzero support is always data[:2, :2], so only
    the top-left 2x2 of data and of each dw_k ever contribute, and the final
    d.mean() is sum(data[:2, :2]) / 49.
  * mean(softmax(x)) over n elements is exactly 1/n (softmax sums to 1), so
    the attn branch is the constant A = (1/4 + 1/9 + 1/25 + 1/49) / 4.

So:  result = (A/4) * sum_i sigmoid(s_i * pw_w[i] + pw_b[i]) + (A/49) * sum(d2)
with s_i = <data[:2,:2], dw_k[:2,:2]> and d2 = data[:2,:2].

Device kernel (replicated on all 8 cores; the op is scalar-sized so there is
nothing to shard): the host packs all operands and constants into a single
[8, 12] f32 buffer so the device needs exactly one input DMA:

    row i in 0..3:  [dw_i 2x2 (4) | d2 (4) | pw_w[i] | pw_b[i] | 0    | A/4 ]
    row 4+j:        [0 ...                                      | d2_j | A/49]

  1. tensor_tensor_reduce (DVE): s[4,1] = sum_x(T[0:4,0:4] * T[0:4,4:8])
  2. activation Sigmoid (ACT):   T[0:4,10] = sigmoid(s * pw_w + pw_b)
                                 (per-partition scale/bias APs)
  3. matmul (PE, K=8):           psum[1,1] = T[:,10].T @ T[:,11]
                                 = A/4 * sum(sig) + A/49 * sum(d2)  == result
  4. copy PSUM -> SBUF, DMA out.
"""

import sys

import numpy as np

if "/opt/trn_rl_repo" not in sys.path:
    sys.path.insert(0, "/opt/trn_rl_repo")

import concourse.mybir as mybir
from concourse import bacc, tile
from concourse.bass_utils import run_bass_kernel_spmd

N_CORES = 8
_F32 = mybir.dt.float32

# mean(softmax(x)) over k*k elements == 1/k^2 exactly; mean over the 4 steps.
ATTN_MEAN = (1 / 4 + 1 / 9 + 1 / 25 + 1 / 49) / 4

_NC_CACHE = None


def build_bass():
    nc = bacc.Bacc(None)
    packed = nc.dram_tensor("packed", [8, 12], _F32, kind="ExternalInput")
    out = nc.dram_tensor("out", [1, 1], _F32, kind="ExternalOutput")
    with tile.TileContext(nc) as tc:
        with (
            tc.tile_pool(name="sb", bufs=1) as sb,
            tc.tile_pool(name="ps", bufs=1, space="PSUM") as ps,
        ):
            T = sb.tile([8, 12], _F32)
            prod = sb.tile([4, 4], _F32)
            s = sb.tile([4, 1], _F32)
            res = sb.tile([1, 1], _F32)
            P = ps.tile([1, 1], _F32)

            # Dependency-free sigmoid on a preamble-initialized const AP:
            # pulls the ACT-table load to the head of the Scalar queue so it
            # overlaps the input DMA instead of sitting on the critical path.
            dummy = sb.tile([1, 1], _F32)
            nc.scalar.activation(
                dummy[:, :],
                nc.const_aps.scalar_like(0.0, dummy[:, :]),
                mybir.ActivationFunctionType.Sigmoid,
            )

            nc.sync.dma_start(T[:, :], packed[:, :])
            nc.vector.scalar_tensor_tensor(
                out=prod[:, :],
                in0=T[0:4, 0:4],
                scalar=1.0,
                in1=T[0:4, 4:8],
                op0=mybir.AluOpType.mult,
                op1=mybir.AluOpType.mult,
                accum_out=s[:, :],
            )
            nc.scalar.activation(
                T[0:4, 10:11],
                s[:, :],
                mybir.ActivationFunctionType.Sigmoid,
                bias=T[0:4, 9:10],
                scale=T[0:4, 8:9],
            )
            nc.tensor.matmul(P[:, :], T[:, 10:11], T[:, 11:12], start=True, stop=True)
            nc.vector.tensor_copy(res[:, :], P[:, :])
            nc.sync.dma_start(out[:, :], res[:, :])
    if not nc.is_finalized():
        nc.finalize()
    return nc


def pack_inputs(data, dw2, dw3, dw5, dw7, pw_w, pw_b):
    d2 = np.asarray(data, np.float32)[:2, :2].reshape(-1)
    packed = np.zeros((8, 12), np.float32)
    for i, w in enumerate((dw2, dw3, dw5, dw7)):
        packed[i, 0:4] = np.asarray(w, np.float32)[:2, :2].reshape(-1)
    packed[0:4, 4:8] = d2
    packed[0:4, 8] = np.asarray(pw_w, np.float32)
    packed[0:4, 9] = np.asarray(pw_b, np.float32)
    packed[0:4, 11] = np.float32(ATTN_MEAN / 4)
    packed[4:8, 10] = d2
    packed[4:8, 11] = np.float32(ATTN_MEAN / 49)
    return packed


def run_packed(packed, **spmd_kwargs):
    global _NC_CACHE
    if _NC_CACHE is None:
        _NC_CACHE = build_bass()
    in_maps = [{"packed": packed} for _ in range(N_CORES)]
    return run_bass_kernel_spmd(
        _NC_CACHE, in_maps, core_ids=list(range(N_CORES)), **spmd_kwargs
    )


def kernel(data, dw2, dw3, dw5, dw7, pw_w, pw_b, attn_w):
    packed = pack_inputs(data, dw2, dw3, dw5, dw7, pw_w, pw_b)
    r = run_packed(packed)
    return np.asarray(r.results[0]["out"][0, 0], dtype=np.float32)
